# revision 1
# baseline (speedup 1.0000x reference)
"""Causal GQA self-attention (B=2, S=2048, H=16, HKV=4, D=128) on 8 trn2 cores.

Sharding: tensor-parallel over heads (4-way, Megatron-style) x data-parallel
over batch (2-way). Core (b, g) handles batch b, q-heads 4g..4g+3, kv-head g.
Each core returns a partial output [S, dim]; host sums the 4 TP partials.

Device layout notes:
  - All activations are kept feature-major ("T layout", [feat, seq]) so every
    matmul contraction runs over the 128-partition dim.
  - Host pre-transposes x and the weight shards (pure layout prep, no FLOPs).
  - scores are computed transposed ([sk, sq]); softmax normalization l[sq] is
    accumulated on GPSIMD + a [1,N] ones-matmul, and the divide is fused into
    the PSUM->SBUF copy of Y. Softmax needs no max-subtraction: rms-normed
    q,k bound |scores/sqrt(D)| <= sqrt(D) ~ 11.3.
"""

import os
import sys

import numpy as np

for _p in ("/opt/trn_rl_repo", "/root/.axon_site/_ro/trn_rl_repo"):
    if os.path.isdir(_p) and _p not in sys.path:
        sys.path.insert(0, _p)

import concourse.bass as bass
import concourse.bacc as bacc
import concourse.tile as tile
from concourse import mybir
from concourse.bass_utils import run_bass_kernel_spmd
from concourse.masks import make_identity
import ml_dtypes

F32 = mybir.dt.float32
F32R = mybir.dt.float32r
BF16 = mybir.dt.bfloat16

B, S, H, HKV, D = 2, 2048, 16, 4, 128
DIM = H * D            # 2048
G = 4                  # TP degree (kv heads)
HQ = H // HKV          # q heads per core = 4
MQ = HQ * D            # 512 q features per core
KM = MQ + 2 * D        # 768 = q(512) + k(128) + v(128) projection rows per core
NKT = DIM // 128       # 16 k-tiles of the contraction dim
NCH = S // 512         # 4 seq chunks of 512
NST = S // 128         # 16 seq tiles of 128
SCALE = float(1.0 / np.sqrt(D))
EPS = float(np.finfo(np.float32).eps)

_CACHED = {}


def _build_program():
    nc = bacc.Bacc("TRN2", target_bir_lowering=False)

    xt_d = nc.dram_tensor("xt", [DIM, S], BF16, kind="ExternalInput")
    wqkv_d = nc.dram_tensor("wqkv", [DIM, KM], BF16, kind="ExternalInput")
    wot_d = nc.dram_tensor("wot", [MQ, DIM], BF16, kind="ExternalInput")
    cosf_d = nc.dram_tensor("cosf", [D, S], F32, kind="ExternalInput")
    sinf_d = nc.dram_tensor("sinf", [D, S], F32, kind="ExternalInput")
    qg_d = nc.dram_tensor("qg", [1, HQ], F32, kind="ExternalInput")
    out_d = nc.dram_tensor("out", [S, DIM], F32, kind="ExternalOutput")

    with tile.TileContext(nc) as tc:
        with tc.tile_pool(name="singles", bufs=1) as singles:
            ident = singles.tile([128, 128], BF16)
            make_identity(nc, ident)
            ones_f = singles.tile([128, 1], F32)
            nc.vector.memset(ones_f, 1.0)
            ones = singles.tile([128, 1], F32R)
            nc.vector.tensor_copy(ones, ones_f)
            ones_row_f = singles.tile([1, 128], F32)
            nc.vector.memset(ones_row_f, 1.0)
            ones_row = singles.tile([1, 128], F32R)
            nc.vector.tensor_copy(ones_row, ones_row_f)
            eps_t = singles.tile([1, 1], F32)
            nc.vector.memset(eps_t, EPS)
            qg_t = singles.tile([1, HQ], F32)
            nc.sync.dma_start(out=qg_t, in_=qg_d[:, :])

            # 4 diagonal causal masks (multiplicative, bf16).
            # mask[delta][p, f] = 1 if f - p - 128*delta >= 0 else 0.
            masks = singles.tile([128, 4, 512], BF16)
            for dlt in range(4):
                m = masks[:, dlt, :]
                nc.gpsimd.memset(m, 1.0)
                nc.gpsimd.affine_select(
                    out=m, in_=m,
                    compare_op=mybir.AluOpType.is_ge,
                    fill=0.0, base=-128 * dlt,
                    pattern=[[1, 512]], channel_multiplier=-1,
                )

            # persistent activations
            qtr = singles.tile([128, HQ, S], BF16)     # roped q, [d, h, s]
            ktr = singles.tile([128, S], BF16)         # roped k, [d, s]
            vtb = singles.tile([128, S], BF16)         # v, [d, s]
            v_nat = singles.tile([128, NST, D], BF16)  # v natural, [sk, t, d]
            yt = singles.tile([128, HQ, S], BF16)      # attn out, [d, h, s]

            # ---------------- projections + rmsnorm + rope ----------------
            with tc.tile_pool(name="proj", bufs=1) as proj, \
                 tc.tile_pool(name="xtp", bufs=2) as xtp, \
                 tc.tile_pool(name="rows", bufs=3) as rows, \
                 tc.tile_pool(name="sqp", bufs=2) as sqp, \
                 tc.tile_pool(name="ropep", bufs=1) as ropep, \
                 tc.tile_pool(name="qfp", bufs=4) as qfp, \
                 tc.tile_pool(name="pps", bufs=1, space="PSUM") as pps:

                w_sb = proj.tile([128, NKT, KM], BF16)
                nc.sync.dma_start(
                    out=w_sb,
                    in_=wqkv_d[:, :].rearrange("(kt p) m -> p kt m", p=128),
                )
                cosf = proj.tile([128, S], F32)
                sinf = proj.tile([128, S], F32)
                nc.sync.dma_start(out=cosf, in_=cosf_d[:, :])
                nc.sync.dma_start(out=sinf, in_=sinf_d[:, :])
                # normed (pre-rope) q/k in f32: [d, head, s]; head 4 = k head
                qn = proj.tile([128, HQ + 1, S], BF16)

                xt_r = xt_d[:, :].rearrange("(kt p) s -> p kt s", p=128)
                for c in range(NCH):
                    xt_c = [
                        xtp.tile([128, 512], BF16, tag=f"xt{kt}",
                                 name=f"xt{kt}")
                        for kt in range(NKT)
                    ]
                    for kt in range(NKT):
                        nc.gpsimd.dma_start(
                            out=xt_c[kt],
                            in_=xt_r[:, kt, 512 * c:512 * (c + 1)],
                        )
                    ps = [pps.tile([128, 512], F32, tag=f"pp{m}", name=f"pp{m}") for m in range(6)]
                    for kt in range(NKT):
                        for m in range(6):
                            nc.tensor.matmul(
                                ps[m],
                                w_sb[:, kt, 128 * m:128 * (m + 1)],
                                xt_c[kt],
                                start=(kt == 0), stop=(kt == NKT - 1),
                            )
                    # v: straight to bf16 sbuf
                    nc.scalar.copy(vtb[:, 512 * c:512 * (c + 1)], ps[5])
                    # q heads + k head: rmsnorm via sumsq(ones-matmul) fused
                    for m in range(5):
                        qf = qfp.tile([128, 512], F32, tag="qf")
                        nc.scalar.copy(qf, ps[m])
                        sq = sqp.tile([128, 512], F32R, tag="sq")
                        nc.vector.tensor_mul(sq, qf, qf)
                        ssq = pps.tile([1, 512], F32, tag="ssq", bufs=1)
                        nc.tensor.matmul(ssq, ones, sq, start=True, stop=True)
                        row = rows.tile([1, 512], F32, tag="row")
                        nc.scalar.activation(
                            out=row, in_=ssq,
                            func=mybir.ActivationFunctionType.Sqrt,
                            bias=eps_t[:, :], scale=1.0 / D,
                        )
                        nc.vector.reciprocal(row, row)
                        rowr = rows.tile([1, 512], F32R, tag="rowr")
                        if m < HQ:
                            nc.vector.tensor_scalar_mul(
                                rowr, row, qg_t[0:1, m:m + 1])
                        else:
                            nc.vector.tensor_copy(rowr, row)
                        rb = pps.tile([128, 512], F32, tag="rbp", name="rb")
                        nc.tensor.matmul(
                            rb, ones_row, rowr, start=True, stop=True,
                        )
                        nc.vector.tensor_mul(
                            qn[:, m, 512 * c:512 * (c + 1)], qf, rb)

                # rope: out = qn*cosf + rot(qn)*sinf  (sinf sign-baked);
                # rot swaps the d-halves, done with two SBUF->SBUF DMAs.
                for m in range(5):
                    dst = ktr if m == HQ else qtr[:, m, :]
                    rot = ropep.tile([128, S], BF16, tag="rot")
                    nc.gpsimd.dma_start(out=rot[0:64, :],
                                        in_=qn[64:128, m, :])
                    nc.gpsimd.dma_start(out=rot[64:128, :],
                                        in_=qn[0:64, m, :])
                    m1 = ropep.tile([128, S], F32, tag="m1")
                    m2 = ropep.tile([128, S], F32, tag="m2")
                    nc.vector.tensor_mul(m1, qn[:, m, :], cosf)
                    nc.vector.tensor_mul(m2, rot, sinf)
                    nc.vector.tensor_add(dst, m1, m2)

            # ---------------- v transpose to natural layout ----------------
            with tc.tile_pool(name="vt_ps", bufs=4, space="PSUM") as vt_ps:
                for t in range(NST):
                    pv = vt_ps.tile([128, 128], BF16, tag="pv")
                    nc.tensor.transpose(
                        pv, vtb[:, 128 * t:128 * (t + 1)], ident)
                    nc.scalar.copy(v_nat[:, t, :], pv)

            # ---------------- attention ----------------
            with tc.tile_pool(name="att", bufs=4) as att, \
                 tc.tile_pool(name="attl", bufs=3) as attl, \
                 tc.tile_pool(name="ps_s", bufs=3, space="PSUM") as ps_s, \
                 tc.tile_pool(name="ps_y", bufs=2, space="PSUM") as ps_y, \
                 tc.tile_pool(name="ps_l", bufs=1, space="PSUM") as ps_l:
                for h in range(HQ):
                    for c in range(NCH):
                        nt = 4 * c + 4  # causal: sk tiles 0..4c+3
                        py = ps_y.tile([128, 512], F32, tag="py")
                        lacc = attl.tile([128, 512], F32R, tag="lacc")
                        for t in range(nt):
                            pscore = ps_s.tile([128, 512], F32, tag="pscore")
                            nc.tensor.matmul(
                                pscore,
                                ktr[:, 128 * t:128 * (t + 1)],
                                qtr[:, h, 512 * c:512 * (c + 1)],
                                start=True, stop=True,
                            )
                            pt = att.tile([128, 512], BF16, tag="pt")
                            nc.scalar.activation(
                                out=pt, in_=pscore,
                                func=mybir.ActivationFunctionType.Exp,
                                scale=SCALE,
                            )
                            if t >= 4 * c:
                                nc.gpsimd.tensor_mul(
                                    pt, pt, masks[:, t - 4 * c, :])
                            if t == 0:
                                nc.gpsimd.tensor_copy(lacc, pt)
                            else:
                                nc.gpsimd.tensor_add(lacc, lacc, pt)
                            nc.tensor.matmul(
                                py, v_nat[:, t, :], pt,
                                start=(t == 0), stop=(t == nt - 1),
                            )
                        pl = ps_l.tile([1, 512], F32, tag="pl")
                        nc.tensor.matmul(pl, ones, lacc, start=True, stop=True)
                        lrow = attl.tile([1, 512], F32R, tag="lrow")
                        nc.scalar.copy(lrow, pl)
                        lb = ps_l.tile([128, 512], F32, tag="lb", bufs=2)
                        nc.tensor.matmul(
                            lb, ones_row, lrow, start=True, stop=True,
                        )
                        rbs = attl.tile([128, 512], F32, tag="rbs")
                        nc.vector.reciprocal(rbs, lb)
                        nc.vector.tensor_mul(
                            yt[:, h, 512 * c:512 * (c + 1)], py, rbs)

            # ---------------- output projection ----------------
            with tc.tile_pool(name="wop", bufs=1) as wop, \
                 tc.tile_pool(name="otp", bufs=4) as otp, \
                 tc.tile_pool(name="ps_o", bufs=4, space="PSUM") as ps_o:
                wo_sb = wop.tile([128, HQ, DIM], BF16)
                nc.gpsimd.dma_start(
                    out=wo_sb,
                    in_=wot_d[:, :].rearrange("(h p) j -> p h j", p=128),
                )
                for st in range(NST):
                    for jc in range(NCH):
                        po = ps_o.tile([128, 512], F32, tag="po")
                        for h in range(HQ):
                            nc.tensor.matmul(
                                po,
                                yt[:, h, 128 * st:128 * (st + 1)],
                                wo_sb[:, h, 512 * jc:512 * (jc + 1)],
                                start=(h == 0), stop=(h == HQ - 1),
                            )
                        ot = otp.tile([128, 512], F32, tag="ot")
                        nc.scalar.copy(ot, po)
                        nc.gpsimd.dma_start(
                            out=out_d[128 * st:128 * (st + 1),
                                      512 * jc:512 * (jc + 1)],
                            in_=ot,
                        )
    nc.compile()
    return nc


def _rope_tables():
    inv_freq = 1.0 / (10000.0 ** (np.arange(0, D, 2, dtype=np.float32) / D))
    t = np.arange(S, dtype=np.float32)
    freqs = np.outer(t, inv_freq)          # [S, 64] f32
    cos = np.cos(freqs).T                  # [64, S]
    sin = np.sin(freqs).T
    cosf = np.concatenate([cos, cos], 0).astype(np.float32)    # [128, S]
    sinf = np.concatenate([sin, -sin], 0).astype(np.float32)
    return np.ascontiguousarray(cosf), np.ascontiguousarray(sinf)


def kernel(x, wq, wk, wv, wo, q_gain):
    x = np.asarray(x, dtype=np.float32)
    wq = np.asarray(wq, dtype=np.float32)
    wk = np.asarray(wk, dtype=np.float32)
    wv = np.asarray(wv, dtype=np.float32)
    wo = np.asarray(wo, dtype=np.float32)
    q_gain = np.asarray(q_gain, dtype=np.float32)

    if "nc" not in _CACHED:
        _CACHED["nc"] = _build_program()
    nc = _CACHED["nc"]

    cosf, sinf = _rope_tables()
    bf = ml_dtypes.bfloat16

    in_maps = []
    for core in range(8):
        b, g = divmod(core, G)
        xt = np.ascontiguousarray(x[b].T).astype(bf)
        wqkv = np.ascontiguousarray(
            np.concatenate(
                [wq[MQ * g:MQ * (g + 1)],
                 wk[D * g:D * (g + 1)],
                 wv[D * g:D * (g + 1)]], axis=0).T).astype(bf)
        wot = np.ascontiguousarray(wo[:, MQ * g:MQ * (g + 1)].T)
        qg = np.ascontiguousarray(q_gain[HQ * g:HQ * (g + 1)].reshape(1, HQ))
        in_maps.append({
            "xt": xt, "wqkv": wqkv, "wot": wot.astype(bf),
            "cosf": cosf, "sinf": sinf, "qg": qg.astype(np.float32),
        })

    res = run_bass_kernel_spmd(nc, in_maps, core_ids=list(range(8)))
    outs = res.results

    y = np.empty((B, S, DIM), dtype=np.float32)
    for b in range(B):
        acc = np.zeros((S, DIM), dtype=np.float64)
        for g in range(G):
            acc += outs[G * b + g]["out"].astype(np.float64)
        y[b] = acc.astype(np.float32)
    return y

